# revision 19
# baseline (speedup 1.0000x reference)
"""BERT self-attention on 8 trn2 NeuronCores.

Sharding: DP over batch (4) x TP over heads (2 groups of 8 heads) = 8 cores.
Each core receives hidden[b].T ([D, S], bf16) plus its head-group's slices of
Wq/Wk/Wv (bf16) and produces the [S, 512] f32 context slice for
(batch b, heads 8g..8g+7).  No collectives; host scatters/gathers.

Per-core plan (bf16 matmuls, f32 PSUM accumulation):
  - qT, kT in [dim, tok] layout; v in [tok, dim] layout with an extra ones
    column per head (softmax denominator rides along in the ctx matmul).
  - scores^T = K @ Q^T with k-tokens on partitions (2 heads packed onto the
    two 64-row halves of the PE array -> concurrent matmuls); exp on ScalarE
    with mask as per-partition bias and scale=1/8 fused, FD=1024 per
    ACTIVATE; ctx[q, hd+1] = probs^T.T @ v_aug accumulated over k-chunks;
    reciprocal + per-partition multiply as epilogue.
  - ScalarE's exp stream is the bottleneck (~261us); the kernel is ordered
    so exp starts as early as possible: only group 0's k/q projections go
    up front, V follows, and groups 1-3's k/q projections are spread as
    background PE work inside earlier attention iterations.
"""

import numpy as np

import concourse.bass as bass
from concourse import bacc
import concourse.mybir as mybir
import concourse.tile as tile
from concourse.bass_utils import run_bass_kernel_spmd

B, S, D, H, HD = 4, 2048, 1024, 16, 64
NCORES = 8
GD = 512          # output dims per core (8 heads x 64)
GH = 8            # heads per core
DC = D // 128     # 8 d-chunks
KC = S // 128     # 16 k-token chunks
QT = S // 512     # 4 q-tiles of 512
SCALE = 1.0 / 8.0  # 1/sqrt(HD)

F32 = mybir.dt.float32
BF16 = mybir.dt.bfloat16


def _emit(tc, ht_d, wq_d, wk_d, wv_d, bq_d, bk_d, bv_d, mask_d, out_d):
    from contextlib import ExitStack

    nc = tc.nc
    with ExitStack() as ctx:
        const = ctx.enter_context(tc.tile_pool(name="const", bufs=1))
        persist = ctx.enter_context(tc.tile_pool(name="persist", bufs=1))
        probs = ctx.enter_context(tc.tile_pool(name="probs", bufs=3))
        outp = ctx.enter_context(tc.tile_pool(name="outp", bufs=1))
        small = ctx.enter_context(tc.tile_pool(name="small", bufs=4))

        # ---- input DMAs (all bf16 except mask/bq/bk), k/q weights first ----
        wk_bf = persist.tile([128, DC, GD], BF16, name="wk_bf")
        wq_bf = persist.tile([128, DC, GD], BF16, name="wq_bf")
        ht_bf = persist.tile([128, DC, S], BF16, name="ht_bf")
        # wk+ht gate the first projection tile -> first exp; wq streams next
        for d in range(DC):
            nc.sync.dma_start(out=wk_bf[:, d, :],
                              in_=wk_d[d * 128:(d + 1) * 128, :])
            nc.sync.dma_start(out=ht_bf[:, d, :],
                              in_=ht_d[d * 128:(d + 1) * 128, :])
        for d in range(DC):
            nc.sync.dma_start(out=wq_bf[:, d, :],
                              in_=wq_d[d * 128:(d + 1) * 128, :])
        wv_bf = persist.tile([128, DC, GD], BF16, name="wv_bf")
        nc.sync.dma_start(out=wv_bf, in_=wv_d[:].rearrange("(c p) j -> p c j", p=128))

        mask_sb = const.tile([128, KC], F32, name="mask_sb")
        nc.sync.dma_start(out=mask_sb, in_=mask_d[:])
        bq_sb = const.tile([128, 4], F32, name="bq_sb")
        nc.sync.dma_start(out=bq_sb, in_=bq_d[:])
        bk_sb = const.tile([128, 4], F32, name="bk_sb")
        nc.sync.dma_start(out=bk_sb, in_=bk_d[:])
        bv_sb = const.tile([1, GD], BF16, name="bv_sb")
        nc.sync.dma_start(out=bv_sb, in_=bv_d[:])
        ones_sb = const.tile([1, 128], BF16, name="ones_sb")
        nc.vector.memset(ones_sb, 1.0)

        # persistent activations
        kT = persist.tile([128, 4, S], BF16, name="kT")   # [dim-in-group, g, tok]
        qT = persist.tile([128, 4, S], BF16, name="qT")
        v_sb = persist.tile([128, KC, GH, HD + 1], BF16, name="v_sb")
        nc.vector.memset(v_sb, 1.0)  # ones column at [..., 64] survives
        # probs for (g0, qt0) are held here; its ctx matmuls run as
        # background work inside group 3 (v isn't ready during qt0).
        pr_hold = persist.tile([128, KC, 1024], BF16, name="pr_hold")

        psum = ctx.enter_context(tc.tile_pool(name="psum", bufs=2, space="PSUM"))

        def emit_kq_tile(which, g, t):
            """Project one [128, 512] tile of kT or qT (group g, token tile
            t).  Returns a list of closures, each emitting one instruction."""
            wbf, dst, bias = (
                (wk_bf, kT, bk_sb) if which == "k" else (wq_bf, qT, bq_sb))
            ps = [None]

            def mk_mm(d):
                def go():
                    if d == 0:
                        ps[0] = psum.tile([128, 512], F32, tag="proj",
                                          name="ps_proj")
                    nc.tensor.matmul(
                        ps[0],
                        wbf[:, d, g * 128:(g + 1) * 128],
                        ht_bf[:, d, t * 512:(t + 1) * 512],
                        start=(d == 0), stop=(d == DC - 1))
                return go

            def fin():
                nc.vector.tensor_scalar_add(
                    out=dst[:, g, t * 512:(t + 1) * 512],
                    in0=ps[0], scalar1=bias[:, g:g + 1])
            return [mk_mm(d) for d in range(DC)] + [fin]

        def emit_v_chunk(c):
            """Project v token-chunk c ([128, 512] + bias row), strided into
            the ones-augmented layout."""
            ps = psum.tile([128, GD], F32, tag="proj", name="ps_v")
            for d in range(DC):
                nc.tensor.matmul(
                    ps, ht_bf[:, d, c * 128:(c + 1) * 128], wv_bf[:, d, :],
                    start=(d == 0), stop=False)
            nc.tensor.matmul(ps, ones_sb, bv_sb, start=False, stop=True)
            nc.vector.tensor_copy(
                out=v_sb[:, c, :, 0:HD],
                in_=ps.rearrange("p (h j) -> p h j", h=GH))

        # Only the first k/q tiles of group 0 go up front; the exp stream
        # starts right after on (g0, qt0) scores while the remaining group-0
        # projections stream in as background work; v follows.
        for f in emit_kq_tile("k", 0, 0):
            f()
        for f in emit_kq_tile("q", 0, 0):
            f()


        # ---- (g0, qt0): scores + exp only; ctx deferred to group 3 ----
        with tc.tile_pool(name="sc0_psum", bufs=2, space="PSUM") as sc0_psum:
            bg0 = []
            for t in range(1, 4):
                bg0.extend(emit_kq_tile("k", 0, t))
            for t in range(1, 4):
                bg0.extend(emit_kq_tile("q", 0, t))
            bg0_i = [0]
            for c in range(KC):
                sc = sc0_psum.tile([128, 1024], F32, tag="sc", name="sc")
                for h01 in range(2):
                    nc.tensor.matmul(
                        sc[:, h01 * 512:(h01 + 1) * 512],
                        kT[h01 * 64:(h01 + 1) * 64, 0, c * 128:(c + 1) * 128],
                        qT[h01 * 64:(h01 + 1) * 64, 0, 0:512],
                        start=True, stop=True)
                nc.scalar.activation(
                    out=pr_hold[:, c, :], in_=sc,
                    func=mybir.ActivationFunctionType.Exp,
                    bias=mask_sb[:, c:c + 1], scale=SCALE)
                for _ in range(4):
                    if bg0_i[0] < len(bg0):
                        bg0[bg0_i[0]]()
                        bg0_i[0] += 1
            while bg0_i[0] < len(bg0):
                bg0[bg0_i[0]]()
                bg0_i[0] += 1
        for c in range(KC):
            emit_v_chunk(c)

        def deferred_qt0_items(out_t0, dctx):
            """ctx + epilogue for (g0, qt0), consuming pr_hold and the two
            proj-pool psum slots (free during group 3)."""
            items = []

            def mk_mm(c, h01, s_):
                def go():
                    if c == 0 and s_ == 0:
                        dctx[h01] = psum.tile([128, 512], F32, tag="proj",
                                              name="dctx")
                    nc.tensor.matmul(
                        dctx[h01][:, s_ * 65:s_ * 65 + 65],
                        pr_hold[:, c, h01 * 512 + s_ * 128:
                                h01 * 512 + (s_ + 1) * 128],
                        v_sb[:, c, h01, :],
                        start=(c == 0 and s_ == 0),
                        stop=(c == KC - 1 and s_ == 3))
                return go

            for c in range(KC):
                for h01 in range(2):
                    for s_ in range(4):
                        items.append(mk_mm(c, h01, s_))

            def mk_epi(h01):
                def go():
                    cps = dctx[h01][:, 0:4 * (HD + 1)]
                    rec = small.tile([128, 4], F32, tag="rec", name="rec")
                    nc.vector.reciprocal(
                        rec,
                        cps.rearrange("p (s x) -> p s x", x=HD + 1)[:, :, HD])
                    for s_ in range(4):
                        nc.vector.tensor_scalar_mul(
                            out=out_t0[:, s_, h01 * HD:(h01 + 1) * HD],
                            in0=cps[:, s_ * 65:s_ * 65 + HD],
                            scalar1=rec[:, s_:s_ + 1])
                return go
            items.extend([mk_epi(0), mk_epi(1)])

            def dma_out0():
                nc.sync.dma_start(
                    out=out_d[:].rearrange("(t s p) j -> t p s j",
                                           s=4, p=128)[0],
                    in_=out_t0)
            items.append(dma_out0)
            return items

        def bg_for_group(g, out_ts, dctx):
            items = []
            if g < 3:
                for which in ("k", "q"):
                    for t in range(4):
                        items.extend(emit_kq_tile(which, g + 1, t))
            else:
                items.extend(deferred_qt0_items(out_ts[0], dctx))
            return items

        # ---- attention ----
        with (
            tc.tile_pool(name="sc_psum", bufs=2, space="PSUM") as sc_psum,
            tc.tile_pool(name="ctx_psum", bufs=1, space="PSUM") as ctx_psum,
        ):
            out_view = out_d[:].rearrange("(t s p) j -> t p s j", s=4, p=128)
            out_ts = {}
            for qt in range(QT):
                out_ts[qt] = outp.tile([128, 4, GD], F32,
                                       tag=f"out{qt}", name="out_t")
            dctx = [None, None]
            for g in range(4):
                bg = bg_for_group(g, out_ts, dctx)
                bg_i = [0]
                qts = range(1, QT) if g == 0 else range(QT)
                n_steps = [len(qts) * KC]

                def pop_bg():
                    # aim to finish the queue ~8 steps before the group ends
                    n_steps[0] -= 1
                    eff = max(n_steps[0] - 8, 1) if n_steps[0] > 0 else 0
                    want = len(bg) - bg_i[0] if eff == 0 else (
                        (len(bg) - bg_i[0] + eff - 1) // eff)
                    for _ in range(want):
                        if bg_i[0] < len(bg):
                            bg[bg_i[0]]()
                            bg_i[0] += 1

                for qt in qts:
                    out_t = out_ts[qt]
                    ctx0 = ctx_psum.tile([128, 4 * (HD + 1)], F32, tag="ctx0",
                                         name="ctx0")
                    ctx1 = ctx_psum.tile([128, 4 * (HD + 1)], F32, tag="ctx1",
                                         name="ctx1")
                    ctxs = (ctx0, ctx1)
                    prs = [None] * KC

                    def emit_ctx(c):
                        for h01 in range(2):
                            cps = ctxs[h01]
                            for s_ in range(4):
                                nc.tensor.matmul(
                                    cps[:, s_ * 65:s_ * 65 + 65],
                                    prs[c][:, h01 * 512 + s_ * 128:
                                           h01 * 512 + (s_ + 1) * 128],
                                    v_sb[:, c, 2 * g + h01, :],
                                    start=(c == 0 and s_ == 0),
                                    stop=(c == KC - 1 and s_ == 3))

                    for c in range(KC):
                        sc = sc_psum.tile([128, 1024], F32, tag="sc", name="sc")
                        for h01 in range(2):
                            nc.tensor.matmul(
                                sc[:, h01 * 512:(h01 + 1) * 512],
                                kT[h01 * 64:(h01 + 1) * 64, g,
                                   c * 128:(c + 1) * 128],
                                qT[h01 * 64:(h01 + 1) * 64, g,
                                   qt * 512:(qt + 1) * 512],
                                start=True, stop=True)
                        pr = probs.tile([128, 1024], BF16, tag="pr", name="pr")
                        nc.scalar.activation(
                            out=pr, in_=sc,
                            func=mybir.ActivationFunctionType.Exp,
                            bias=mask_sb[:, c:c + 1], scale=SCALE)
                        prs[c] = pr
                        if c >= 1:
                            emit_ctx(c - 1)  # overlap ctx(c-1) with exp(c)
                        pop_bg()
                    emit_ctx(KC - 1)

                    for h01 in range(2):
                        cps = ctxs[h01]
                        rec = small.tile([128, 4], F32, tag="rec", name="rec")
                        nc.vector.reciprocal(
                            rec,
                            cps.rearrange("p (s x) -> p s x", x=HD + 1)[:, :, HD])
                        for s_ in range(4):
                            nc.vector.tensor_scalar_mul(
                                out=out_t[:, s_,
                                          (2 * g + h01) * HD:
                                          (2 * g + h01 + 1) * HD],
                                in0=cps[:, s_ * 65:s_ * 65 + HD],
                                scalar1=rec[:, s_:s_ + 1])
                    if g == 3:
                        nc.sync.dma_start(out=out_view[qt], in_=out_ts[qt])
                while bg_i[0] < len(bg):  # flush
                    bg[bg_i[0]]()
                    bg_i[0] += 1


def _build():
    nc = bacc.Bacc()
    ht_d = nc.declare_dram_parameter("ht", [D, S], BF16, isOutput=False)
    wq_d = nc.declare_dram_parameter("wq", [D, GD], BF16, isOutput=False)
    wk_d = nc.declare_dram_parameter("wk", [D, GD], BF16, isOutput=False)
    wv_d = nc.declare_dram_parameter("wv", [D, GD], BF16, isOutput=False)
    bq_d = nc.declare_dram_parameter("bq", [128, 4], F32, isOutput=False)
    bk_d = nc.declare_dram_parameter("bk", [128, 4], F32, isOutput=False)
    bv_d = nc.declare_dram_parameter("bv", [1, GD], BF16, isOutput=False)
    mask_d = nc.declare_dram_parameter("mask", [128, KC], F32, isOutput=False)
    out_d = nc.declare_dram_parameter("out", [S, GD], F32, isOutput=True)
    with tile.TileContext(nc) as tc:
        _emit(tc, ht_d, wq_d, wk_d, wv_d, bq_d, bk_d, bv_d, mask_d, out_d)
    nc.compile()
    return nc


_NC = None


def _patch_ldw_opt():
    """Flip walrus's --enable-ldw-opt to true (experimental: overlaps
    LDWEIGHTS with matmuls).  Opt-in via BASS_LDW_OPT=1."""
    import os
    if os.environ.get("BASS_LDW_OPT") != "1":
        return
    import concourse.bass_utils as bu
    if getattr(bu, "_ldw_patched", False):
        return
    orig = bu.run_command

    def patched(argv, **kwargs):
        argv = ["--enable-ldw-opt=true" if a == "--enable-ldw-opt=false" else a
                for a in argv]
        return orig(argv, **kwargs)

    bu.run_command = patched
    bu._ldw_patched = True


def _get_nc():
    global _NC
    if _NC is None:
        _patch_ldw_opt()
        _NC = _build()
    return _NC


def _prep_in_maps(hidden_states, attention_mask, Wq, bq, Wk, bk, Wv, bv):
    import ml_dtypes
    bf16 = ml_dtypes.bfloat16

    hs = np.asarray(hidden_states, dtype=np.float32)
    am = np.asarray(attention_mask, dtype=np.float32)
    Wq = np.asarray(Wq, dtype=np.float32)
    Wk = np.asarray(Wk, dtype=np.float32)
    Wv = np.asarray(Wv, dtype=np.float32)
    bq = np.asarray(bq, dtype=np.float32)
    bk = np.asarray(bk, dtype=np.float32)
    bv = np.asarray(bv, dtype=np.float32)

    hts = [np.ascontiguousarray(hs[b].T).astype(bf16) for b in range(B)]
    masks = [np.ascontiguousarray(am[b, 0, 0].reshape(KC, 128).T)
             for b in range(B)]
    in_maps = []
    for c in range(NCORES):
        b, g = divmod(c, 2)
        sl = slice(g * GD, (g + 1) * GD)
        in_maps.append({
            "ht": hts[b],
            "wq": np.ascontiguousarray(Wq[:, sl]).astype(bf16),
            "wk": np.ascontiguousarray(Wk[:, sl]).astype(bf16),
            "wv": np.ascontiguousarray(Wv[:, sl]).astype(bf16),
            "bq": np.ascontiguousarray(bq[sl].reshape(4, 128).T),
            "bk": np.ascontiguousarray(bk[sl].reshape(4, 128).T),
            "bv": np.ascontiguousarray(bv[sl].reshape(1, GD)).astype(bf16),
            "mask": masks[b],
        })
    return in_maps


def _install_trace_hooks():
    """Make trace=True work in this container: register the NTFF profile
    hook under the name bass_utils imports, and keep artifacts local."""
    import sys
    import types

    if "antenv.axon_hooks" not in sys.modules:
        mod = types.ModuleType("antenv.axon_hooks")
        mod._hook = None

        def set_axon_ntff_profile_hook(h):
            mod._hook = h

        def get_axon_ntff_profile_hook():
            return mod._hook

        mod.set_axon_ntff_profile_hook = set_axon_ntff_profile_hook
        mod.get_axon_ntff_profile_hook = get_axon_ntff_profile_hook
        sys.modules["antenv.axon_hooks"] = mod
        try:
            from trn_agent_boot.trn_boot import _ntff_profile_via_ctypes
            mod._hook = _ntff_profile_via_ctypes("/opt/axon/libaxon_pjrt.so")
        except Exception as e:  # profiling degrades, run still works
            print(f"ntff hook install failed: {e}")
    import concourse.bass_utils as bu
    bu.upload_artifacts = lambda tmpdir: tmpdir


def run(inputs, trace=False, trace_cores=None):
    """Run the SPMD kernel.  Returns (full_output, exec_time_ns_or_None)."""
    if trace:
        _install_trace_hooks()
    nc = _get_nc()
    in_maps = _prep_in_maps(**inputs)
    res = run_bass_kernel_spmd(
        nc, in_maps, core_ids=list(range(NCORES)), trace=trace,
        **({"trace_cores": trace_cores} if trace_cores is not None else {}),
    )
    out = np.empty((B, S, D), np.float32)
    for c in range(NCORES):
        b, g = divmod(c, 2)
        out[b, :, g * GD:(g + 1) * GD] = res.results[c]["out"]
    return out, res.exec_time_ns


def kernel(hidden_states, attention_mask, Wq, bq, Wk, bk, Wv, bv):
    out, _ = run(dict(
        hidden_states=hidden_states, attention_mask=attention_mask,
        Wq=Wq, bq=bq, Wk=Wk, bk=bk, Wv=Wv, bv=bv,
    ))
    return out


# revision 21
# speedup vs baseline: 1.0322x; 1.0322x over previous
"""BERT self-attention on 8 trn2 NeuronCores.

Sharding: DP over batch (4) x TP over heads (2 groups of 8 heads) = 8 cores.
Each core receives hidden[b].T ([D, S], bf16) plus its head-group's slices of
Wq/Wk/Wv (bf16) and produces the [S, 512] f32 context slice for
(batch b, heads 8g..8g+7).  No collectives; host scatters/gathers.

Per-core plan (bf16 matmuls, f32 PSUM accumulation):
  - qT, kT in [dim, tok] layout; v in [tok, dim] layout with an extra ones
    column per head (softmax denominator rides along in the ctx matmul).
  - scores^T = K @ Q^T with k-tokens on partitions (2 heads packed onto the
    two 64-row halves of the PE array -> concurrent matmuls); exp on ScalarE
    with mask as per-partition bias and scale=1/8 fused, FD=1024 per
    ACTIVATE; ctx[q, hd+1] = probs^T.T @ v_aug accumulated over k-chunks;
    reciprocal + per-partition multiply as epilogue.
  - ScalarE's exp stream is the bottleneck (~261us); the kernel is ordered
    so exp starts as early as possible: only group 0's k/q projections go
    up front, V follows, and groups 1-3's k/q projections are spread as
    background PE work inside earlier attention iterations.
"""

import numpy as np

import concourse.bass as bass
from concourse import bacc
import concourse.mybir as mybir
import concourse.tile as tile
from concourse.bass_utils import run_bass_kernel_spmd

B, S, D, H, HD = 4, 2048, 1024, 16, 64
NCORES = 8
GD = 512          # output dims per core (8 heads x 64)
GH = 8            # heads per core
DC = D // 128     # 8 d-chunks
KC = S // 128     # 16 k-token chunks
QT = S // 512     # 4 q-tiles of 512
SCALE = 1.0 / 8.0  # 1/sqrt(HD)

F32 = mybir.dt.float32
BF16 = mybir.dt.bfloat16


def _emit(tc, ht_d, wq_d, wk_d, wv_d, bq_d, bk_d, bv_d, mask_d, out_d):
    from contextlib import ExitStack

    nc = tc.nc
    with ExitStack() as ctx:
        const = ctx.enter_context(tc.tile_pool(name="const", bufs=1))
        persist = ctx.enter_context(tc.tile_pool(name="persist", bufs=1))
        probs = ctx.enter_context(tc.tile_pool(name="probs", bufs=3))
        outp = ctx.enter_context(tc.tile_pool(name="outp", bufs=1))
        small = ctx.enter_context(tc.tile_pool(name="small", bufs=4))

        # ---- input DMAs (all bf16 except mask/bq/bk), per d-chunk so the
        # d-outer preamble matmuls can start as chunks arrive ----
        wk_bf = persist.tile([128, DC, GD], BF16, name="wk_bf")
        wq_bf = persist.tile([128, DC, GD], BF16, name="wq_bf")
        wv_bf = persist.tile([128, DC, GD], BF16, name="wv_bf")
        ht_bf = persist.tile([128, DC, S], BF16, name="ht_bf")
        for d in range(DC):
            nc.sync.dma_start(out=wk_bf[:, d, :],
                              in_=wk_d[d * 128:(d + 1) * 128, :])
            nc.sync.dma_start(out=wq_bf[:, d, :],
                              in_=wq_d[d * 128:(d + 1) * 128, :])
            nc.sync.dma_start(out=wv_bf[:, d, :],
                              in_=wv_d[d * 128:(d + 1) * 128, :])
            nc.sync.dma_start(out=ht_bf[:, d, :],
                              in_=ht_d[d * 128:(d + 1) * 128, :])

        mask_sb = const.tile([128, KC], F32, name="mask_sb")
        nc.sync.dma_start(out=mask_sb, in_=mask_d[:])
        bq_sb = const.tile([128, 4], F32, name="bq_sb")
        nc.sync.dma_start(out=bq_sb, in_=bq_d[:])
        bk_sb = const.tile([128, 4], F32, name="bk_sb")
        nc.sync.dma_start(out=bk_sb, in_=bk_d[:])
        bv_sb = const.tile([1, GD], BF16, name="bv_sb")
        nc.sync.dma_start(out=bv_sb, in_=bv_d[:])
        ones_sb = const.tile([1, 128], BF16, name="ones_sb")
        nc.vector.memset(ones_sb, 1.0)

        # persistent activations
        kT = persist.tile([128, 4, S], BF16, name="kT")   # [dim-in-group, g, tok]
        qT = persist.tile([128, 4, S], BF16, name="qT")
        v_sb = persist.tile([128, KC, GH, HD + 1], BF16, name="v_sb")
        nc.vector.memset(v_sb, 1.0)  # ones column at [..., 64] survives
        # probs for (g0, qt0) are held here; its ctx matmuls run as
        # background work inside group 3 (v isn't ready during qt0).
        pr_hold = persist.tile([128, KC, 1024], BF16, name="pr_hold")

        psum = ctx.enter_context(tc.tile_pool(name="psum", bufs=2, space="PSUM"))

        def emit_kq_tile(which, g, t):
            """Project one [128, 512] tile of kT or qT (group g, token tile
            t).  Returns a list of closures, each emitting one instruction."""
            wbf, dst, bias = (
                (wk_bf, kT, bk_sb) if which == "k" else (wq_bf, qT, bq_sb))
            ps = [None]

            def mk_mm(d):
                def go():
                    if d == 0:
                        ps[0] = psum.tile([128, 512], F32, tag="proj",
                                          name="ps_proj")
                    nc.tensor.matmul(
                        ps[0],
                        wbf[:, d, g * 128:(g + 1) * 128],
                        ht_bf[:, d, t * 512:(t + 1) * 512],
                        start=(d == 0), stop=(d == DC - 1))
                return go

            def fin():
                nc.vector.tensor_scalar_add(
                    out=dst[:, g, t * 512:(t + 1) * 512],
                    in0=ps[0], scalar1=bias[:, g:g + 1])
            return [mk_mm(d) for d in range(DC)] + [fin]

        def v_chunk_items(c):
            """Per-instruction closures projecting v token-chunk c."""
            ps = [None]

            def mk_mm(d):
                def go():
                    if d == 0:
                        ps[0] = psum.tile([128, GD], F32, tag="proj",
                                          name="ps_v")
                    nc.tensor.matmul(
                        ps[0], ht_bf[:, d, c * 128:(c + 1) * 128],
                        wv_bf[:, d, :], start=(d == 0), stop=False)
                return go

            def bias_mm():
                nc.tensor.matmul(ps[0], ones_sb, bv_sb,
                                 start=False, stop=True)

            def copy():
                nc.vector.tensor_copy(
                    out=v_sb[:, c, :, 0:HD],
                    in_=ps[0].rearrange("p (h j) -> p h j", h=GH))
            return [mk_mm(d) for d in range(DC)] + [bias_mm, copy]

        # ---- preamble: k_t0/q_t0 + first 6 v chunks, d-outer so the
        # matmuls pace with the arriving ht/w DMA chunks ----
        VPRE = 6
        kq_k = emit_kq_tile("k", 0, 0)
        kq_q = emit_kq_tile("q", 0, 0)
        with tc.tile_pool(name="vpre", bufs=VPRE, space="PSUM") as vpre:
            vps = [None] * VPRE
            for d in range(DC):
                kq_k[d]()
                kq_q[d]()
                for c in range(VPRE):
                    if d == 0:
                        vps[c] = vpre.tile([128, GD], F32, tag="vpre",
                                           name="vps")
                    nc.tensor.matmul(
                        vps[c], ht_bf[:, d, c * 128:(c + 1) * 128],
                        wv_bf[:, d, :], start=(d == 0), stop=False)
            kq_k[DC]()  # bias-add copies
            kq_q[DC]()
            for c in range(VPRE):
                nc.tensor.matmul(vps[c], ones_sb, bv_sb,
                                 start=False, stop=True)
                nc.vector.tensor_copy(
                    out=v_sb[:, c, :, 0:HD],
                    in_=vps[c].rearrange("p (h j) -> p h j", h=GH))

        # ---- (g0, qt0): scores + exp only; ctx deferred to group 3.
        # Background: remaining kT tiles (needed by qt0's own scores),
        # remaining v chunks, remaining qT tiles. ----
        with tc.tile_pool(name="sc0_psum", bufs=2, space="PSUM") as sc0_psum:
            bg0 = []
            for t in range(1, 4):
                bg0.extend(emit_kq_tile("k", 0, t))
            for c in range(VPRE, KC):
                bg0.extend(v_chunk_items(c))
            for t in range(1, 4):
                bg0.extend(emit_kq_tile("q", 0, t))
            bg0_i = [0]
            for c in range(KC):
                sc = sc0_psum.tile([128, 1024], F32, tag="sc", name="sc")
                for h01 in range(2):
                    nc.tensor.matmul(
                        sc[:, h01 * 512:(h01 + 1) * 512],
                        kT[h01 * 64:(h01 + 1) * 64, 0, c * 128:(c + 1) * 128],
                        qT[h01 * 64:(h01 + 1) * 64, 0, 0:512],
                        start=True, stop=True)
                nc.scalar.activation(
                    out=pr_hold[:, c, :], in_=sc,
                    func=mybir.ActivationFunctionType.Exp,
                    bias=mask_sb[:, c:c + 1], scale=SCALE)
                want = (len(bg0) - bg0_i[0] + (KC - 1 - c)) // max(KC - c, 1)
                for _ in range(want):
                    if bg0_i[0] < len(bg0):
                        bg0[bg0_i[0]]()
                        bg0_i[0] += 1
            while bg0_i[0] < len(bg0):
                bg0[bg0_i[0]]()
                bg0_i[0] += 1

        def deferred_qt0_items(out_t0, dctx):
            """ctx + epilogue for (g0, qt0), consuming pr_hold and the two
            proj-pool psum slots (free during group 3)."""
            items = []

            def mk_mm(c, h01, s_):
                def go():
                    if c == 0 and s_ == 0:
                        dctx[h01] = psum.tile([128, 512], F32, tag="proj",
                                              name="dctx")
                    nc.tensor.matmul(
                        dctx[h01][:, s_ * 65:s_ * 65 + 65],
                        pr_hold[:, c, h01 * 512 + s_ * 128:
                                h01 * 512 + (s_ + 1) * 128],
                        v_sb[:, c, h01, :],
                        start=(c == 0 and s_ == 0),
                        stop=(c == KC - 1 and s_ == 3))
                return go

            for c in range(KC):
                for h01 in range(2):
                    for s_ in range(4):
                        items.append(mk_mm(c, h01, s_))

            def mk_epi(h01):
                def go():
                    cps = dctx[h01][:, 0:4 * (HD + 1)]
                    rec = small.tile([128, 4], F32, tag="rec", name="rec")
                    nc.vector.reciprocal(
                        rec,
                        cps.rearrange("p (s x) -> p s x", x=HD + 1)[:, :, HD])
                    for s_ in range(4):
                        nc.vector.tensor_scalar_mul(
                            out=out_t0[:, s_, h01 * HD:(h01 + 1) * HD],
                            in0=cps[:, s_ * 65:s_ * 65 + HD],
                            scalar1=rec[:, s_:s_ + 1])
                return go
            items.extend([mk_epi(0), mk_epi(1)])

            def dma_out0():
                nc.sync.dma_start(
                    out=out_d[:].rearrange("(t s p) j -> t p s j",
                                           s=4, p=128)[0],
                    in_=out_t0)
            items.append(dma_out0)
            return items

        def bg_for_group(g, out_ts, dctx):
            items = []
            if g < 3:
                for which in ("k", "q"):
                    for t in range(4):
                        items.extend(emit_kq_tile(which, g + 1, t))
            else:
                items.extend(deferred_qt0_items(out_ts[0], dctx))
            return items

        # ---- attention ----
        with (
            tc.tile_pool(name="sc_psum", bufs=2, space="PSUM") as sc_psum,
            tc.tile_pool(name="ctx_psum", bufs=1, space="PSUM") as ctx_psum,
        ):
            out_view = out_d[:].rearrange("(t s p) j -> t p s j", s=4, p=128)
            out_ts = {}
            for qt in range(QT):
                out_ts[qt] = outp.tile([128, 4, GD], F32,
                                       tag=f"out{qt}", name="out_t")
            dctx = [None, None]
            for g in range(4):
                bg = bg_for_group(g, out_ts, dctx)
                bg_i = [0]
                qts = range(1, QT) if g == 0 else range(QT)
                n_steps = [len(qts) * KC]

                def pop_bg():
                    # aim to finish the queue ~8 steps before the group ends
                    n_steps[0] -= 1
                    eff = max(n_steps[0] - 8, 1) if n_steps[0] > 0 else 0
                    want = len(bg) - bg_i[0] if eff == 0 else (
                        (len(bg) - bg_i[0] + eff - 1) // eff)
                    for _ in range(want):
                        if bg_i[0] < len(bg):
                            bg[bg_i[0]]()
                            bg_i[0] += 1

                for qt in qts:
                    out_t = out_ts[qt]
                    ctx0 = ctx_psum.tile([128, 4 * (HD + 1)], F32, tag="ctx0",
                                         name="ctx0")
                    ctx1 = ctx_psum.tile([128, 4 * (HD + 1)], F32, tag="ctx1",
                                         name="ctx1")
                    ctxs = (ctx0, ctx1)
                    prs = [None] * KC

                    def emit_ctx(c):
                        for h01 in range(2):
                            cps = ctxs[h01]
                            for s_ in range(4):
                                nc.tensor.matmul(
                                    cps[:, s_ * 65:s_ * 65 + 65],
                                    prs[c][:, h01 * 512 + s_ * 128:
                                           h01 * 512 + (s_ + 1) * 128],
                                    v_sb[:, c, 2 * g + h01, :],
                                    start=(c == 0 and s_ == 0),
                                    stop=(c == KC - 1 and s_ == 3))

                    for c in range(KC):
                        sc = sc_psum.tile([128, 1024], F32, tag="sc", name="sc")
                        for h01 in range(2):
                            nc.tensor.matmul(
                                sc[:, h01 * 512:(h01 + 1) * 512],
                                kT[h01 * 64:(h01 + 1) * 64, g,
                                   c * 128:(c + 1) * 128],
                                qT[h01 * 64:(h01 + 1) * 64, g,
                                   qt * 512:(qt + 1) * 512],
                                start=True, stop=True)
                        pr = probs.tile([128, 1024], BF16, tag="pr", name="pr")
                        nc.scalar.activation(
                            out=pr, in_=sc,
                            func=mybir.ActivationFunctionType.Exp,
                            bias=mask_sb[:, c:c + 1], scale=SCALE)
                        prs[c] = pr
                        if c >= 1:
                            emit_ctx(c - 1)  # overlap ctx(c-1) with exp(c)
                        pop_bg()
                    emit_ctx(KC - 1)

                    for h01 in range(2):
                        cps = ctxs[h01]
                        rec = small.tile([128, 4], F32, tag="rec", name="rec")
                        nc.vector.reciprocal(
                            rec,
                            cps.rearrange("p (s x) -> p s x", x=HD + 1)[:, :, HD])
                        for s_ in range(4):
                            nc.vector.tensor_scalar_mul(
                                out=out_t[:, s_,
                                          (2 * g + h01) * HD:
                                          (2 * g + h01 + 1) * HD],
                                in0=cps[:, s_ * 65:s_ * 65 + HD],
                                scalar1=rec[:, s_:s_ + 1])
                    if g == 3:
                        nc.sync.dma_start(out=out_view[qt], in_=out_ts[qt])
                while bg_i[0] < len(bg):  # flush
                    bg[bg_i[0]]()
                    bg_i[0] += 1


def _build():
    nc = bacc.Bacc()
    ht_d = nc.declare_dram_parameter("ht", [D, S], BF16, isOutput=False)
    wq_d = nc.declare_dram_parameter("wq", [D, GD], BF16, isOutput=False)
    wk_d = nc.declare_dram_parameter("wk", [D, GD], BF16, isOutput=False)
    wv_d = nc.declare_dram_parameter("wv", [D, GD], BF16, isOutput=False)
    bq_d = nc.declare_dram_parameter("bq", [128, 4], F32, isOutput=False)
    bk_d = nc.declare_dram_parameter("bk", [128, 4], F32, isOutput=False)
    bv_d = nc.declare_dram_parameter("bv", [1, GD], BF16, isOutput=False)
    mask_d = nc.declare_dram_parameter("mask", [128, KC], F32, isOutput=False)
    out_d = nc.declare_dram_parameter("out", [S, GD], F32, isOutput=True)
    with tile.TileContext(nc) as tc:
        _emit(tc, ht_d, wq_d, wk_d, wv_d, bq_d, bk_d, bv_d, mask_d, out_d)
    nc.compile()
    return nc


_NC = None


def _patch_ldw_opt():
    """Flip walrus's --enable-ldw-opt to true (experimental: overlaps
    LDWEIGHTS with matmuls).  Opt-in via BASS_LDW_OPT=1."""
    import os
    if os.environ.get("BASS_LDW_OPT") != "1":
        return
    import concourse.bass_utils as bu
    if getattr(bu, "_ldw_patched", False):
        return
    orig = bu.run_command

    def patched(argv, **kwargs):
        argv = ["--enable-ldw-opt=true" if a == "--enable-ldw-opt=false" else a
                for a in argv]
        return orig(argv, **kwargs)

    bu.run_command = patched
    bu._ldw_patched = True


def _get_nc():
    global _NC
    if _NC is None:
        _patch_ldw_opt()
        _NC = _build()
    return _NC


def _prep_in_maps(hidden_states, attention_mask, Wq, bq, Wk, bk, Wv, bv):
    import ml_dtypes
    bf16 = ml_dtypes.bfloat16

    hs = np.asarray(hidden_states, dtype=np.float32)
    am = np.asarray(attention_mask, dtype=np.float32)
    Wq = np.asarray(Wq, dtype=np.float32)
    Wk = np.asarray(Wk, dtype=np.float32)
    Wv = np.asarray(Wv, dtype=np.float32)
    bq = np.asarray(bq, dtype=np.float32)
    bk = np.asarray(bk, dtype=np.float32)
    bv = np.asarray(bv, dtype=np.float32)

    hts = [np.ascontiguousarray(hs[b].T).astype(bf16) for b in range(B)]
    masks = [np.ascontiguousarray(am[b, 0, 0].reshape(KC, 128).T)
             for b in range(B)]
    in_maps = []
    for c in range(NCORES):
        b, g = divmod(c, 2)
        sl = slice(g * GD, (g + 1) * GD)
        in_maps.append({
            "ht": hts[b],
            "wq": np.ascontiguousarray(Wq[:, sl]).astype(bf16),
            "wk": np.ascontiguousarray(Wk[:, sl]).astype(bf16),
            "wv": np.ascontiguousarray(Wv[:, sl]).astype(bf16),
            "bq": np.ascontiguousarray(bq[sl].reshape(4, 128).T),
            "bk": np.ascontiguousarray(bk[sl].reshape(4, 128).T),
            "bv": np.ascontiguousarray(bv[sl].reshape(1, GD)).astype(bf16),
            "mask": masks[b],
        })
    return in_maps


def _install_trace_hooks():
    """Make trace=True work in this container: register the NTFF profile
    hook under the name bass_utils imports, and keep artifacts local."""
    import sys
    import types

    if "antenv.axon_hooks" not in sys.modules:
        mod = types.ModuleType("antenv.axon_hooks")
        mod._hook = None

        def set_axon_ntff_profile_hook(h):
            mod._hook = h

        def get_axon_ntff_profile_hook():
            return mod._hook

        mod.set_axon_ntff_profile_hook = set_axon_ntff_profile_hook
        mod.get_axon_ntff_profile_hook = get_axon_ntff_profile_hook
        sys.modules["antenv.axon_hooks"] = mod
        try:
            from trn_agent_boot.trn_boot import _ntff_profile_via_ctypes
            mod._hook = _ntff_profile_via_ctypes("/opt/axon/libaxon_pjrt.so")
        except Exception as e:  # profiling degrades, run still works
            print(f"ntff hook install failed: {e}")
    import concourse.bass_utils as bu
    bu.upload_artifacts = lambda tmpdir: tmpdir


def run(inputs, trace=False, trace_cores=None):
    """Run the SPMD kernel.  Returns (full_output, exec_time_ns_or_None)."""
    if trace:
        _install_trace_hooks()
    nc = _get_nc()
    in_maps = _prep_in_maps(**inputs)
    res = run_bass_kernel_spmd(
        nc, in_maps, core_ids=list(range(NCORES)), trace=trace,
        **({"trace_cores": trace_cores} if trace_cores is not None else {}),
    )
    out = np.empty((B, S, D), np.float32)
    for c in range(NCORES):
        b, g = divmod(c, 2)
        out[b, :, g * GD:(g + 1) * GD] = res.results[c]["out"]
    return out, res.exec_time_ns


def kernel(hidden_states, attention_mask, Wq, bq, Wk, bk, Wv, bv):
    out, _ = run(dict(
        hidden_states=hidden_states, attention_mask=attention_mask,
        Wq=Wq, bq=bq, Wk=Wk, bk=bk, Wv=Wv, bv=bv,
    ))
    return out


# revision 26
# speedup vs baseline: 1.0529x; 1.0201x over previous
"""BERT self-attention on 8 trn2 NeuronCores.

Sharding: DP over batch (4) x TP over heads (2 groups of 8 heads) = 8 cores.
Each core receives hidden[b].T ([D, S], bf16) plus its head-group's slices of
Wq/Wk/Wv (bf16) and produces the [S, 512] f32 context slice for
(batch b, heads 8g..8g+7).  No collectives; host scatters/gathers.

Per-core plan (bf16 matmuls, f32 PSUM accumulation):
  - qT, kT in [dim, tok] layout; v in [tok, dim] layout with an extra ones
    column per head (softmax denominator rides along in the ctx matmul).
  - scores^T = K @ Q^T with k-tokens on partitions (2 heads packed onto the
    two 64-row halves of the PE array -> concurrent matmuls); exp on ScalarE
    with mask as per-partition bias and scale=1/8 fused, FD=1024 per
    ACTIVATE; ctx[q, hd+1] = probs^T.T @ v_aug accumulated over k-chunks;
    reciprocal + per-partition multiply as epilogue.
  - ScalarE's exp stream is the bottleneck (~261us); the kernel is ordered
    so exp starts as early as possible: only group 0's k/q projections go
    up front, V follows, and groups 1-3's k/q projections are spread as
    background PE work inside earlier attention iterations.
"""

import numpy as np

import concourse.bass as bass
from concourse import bacc
import concourse.mybir as mybir
import concourse.tile as tile
from concourse.bass_utils import run_bass_kernel_spmd

B, S, D, H, HD = 4, 2048, 1024, 16, 64
NCORES = 8
GD = 512          # output dims per core (8 heads x 64)
GH = 8            # heads per core
DC = D // 128     # 8 d-chunks
KC = S // 128     # 16 k-token chunks
QT = S // 512     # 4 q-tiles of 512
SCALE = 1.0 / 8.0  # 1/sqrt(HD)

F32 = mybir.dt.float32
BF16 = mybir.dt.bfloat16


def _emit(tc, ht_d, wq_d, wk_d, wv_d, bq_d, bk_d, bv_d, mask_d, out_d):
    from contextlib import ExitStack

    nc = tc.nc
    with ExitStack() as ctx:
        const = ctx.enter_context(tc.tile_pool(name="const", bufs=1))
        persist = ctx.enter_context(tc.tile_pool(name="persist", bufs=1))
        probs = ctx.enter_context(tc.tile_pool(name="probs", bufs=3))
        outp = ctx.enter_context(tc.tile_pool(name="outp", bufs=1))
        small = ctx.enter_context(tc.tile_pool(name="small", bufs=4))

        # ---- input DMAs (all bf16 except mask/bq/bk), per d-chunk so the
        # d-outer preamble matmuls can start as chunks arrive ----
        wk_bf = persist.tile([128, DC, GD], BF16, name="wk_bf")
        wq_bf = persist.tile([128, DC, GD], BF16, name="wq_bf")
        wv_bf = persist.tile([128, DC, GD], BF16, name="wv_bf")
        ht_bf = persist.tile([128, DC, S], BF16, name="ht_bf")
        # weights ride the gpsimd (SWDGE) queue, ht the sync (HWDGE) queue,
        # so the two streams run in parallel
        for d in range(DC):
            nc.gpsimd.dma_start(out=wk_bf[:, d, :],
                                in_=wk_d[d * 128:(d + 1) * 128, :])
            nc.gpsimd.dma_start(out=wq_bf[:, d, :],
                                in_=wq_d[d * 128:(d + 1) * 128, :])
            nc.gpsimd.dma_start(out=wv_bf[:, d, :],
                                in_=wv_d[d * 128:(d + 1) * 128, :])
            nc.sync.dma_start(out=ht_bf[:, d, :],
                              in_=ht_d[d * 128:(d + 1) * 128, :])

        mask_sb = const.tile([128, KC], F32, name="mask_sb")
        nc.sync.dma_start(out=mask_sb, in_=mask_d[:])
        bq_sb = const.tile([128, 4], F32, name="bq_sb")
        nc.sync.dma_start(out=bq_sb, in_=bq_d[:])
        bk_sb = const.tile([128, 4], F32, name="bk_sb")
        nc.sync.dma_start(out=bk_sb, in_=bk_d[:])
        bv_sb = const.tile([1, GD], BF16, name="bv_sb")
        nc.sync.dma_start(out=bv_sb, in_=bv_d[:])
        ones_sb = const.tile([1, 128], BF16, name="ones_sb")
        nc.vector.memset(ones_sb, 1.0)

        # persistent activations
        kT = persist.tile([128, 4, S], BF16, name="kT")   # [dim-in-group, g, tok]
        qT = persist.tile([128, 4, S], BF16, name="qT")
        v_sb = persist.tile([128, KC, GH, HD + 1], BF16, name="v_sb")
        nc.vector.memset(v_sb, 1.0)  # ones column at [..., 64] survives
        # probs for (g0, qt0) are held here; its ctx matmuls run as
        # background work inside group 3 (v isn't ready during qt0).
        pr_hold = persist.tile([128, KC, 1024], BF16, name="pr_hold")

        psum = ctx.enter_context(tc.tile_pool(name="psum", bufs=2, space="PSUM"))

        def emit_kq_tile(which, g, t):
            """Project one [128, 512] tile of kT or qT (group g, token tile
            t).  Returns a list of closures, each emitting one instruction."""
            wbf, dst, bias = (
                (wk_bf, kT, bk_sb) if which == "k" else (wq_bf, qT, bq_sb))
            ps = [None]

            def mk_mm(d):
                def go():
                    if d == 0:
                        ps[0] = psum.tile([128, 512], F32, tag="proj",
                                          name="ps_proj")
                    nc.tensor.matmul(
                        ps[0],
                        wbf[:, d, g * 128:(g + 1) * 128],
                        ht_bf[:, d, t * 512:(t + 1) * 512],
                        start=(d == 0), stop=(d == DC - 1))
                return go

            def fin():
                nc.vector.tensor_scalar_add(
                    out=dst[:, g, t * 512:(t + 1) * 512],
                    in0=ps[0], scalar1=bias[:, g:g + 1])
            return [mk_mm(d) for d in range(DC)] + [fin]

        def v_chunk_items(c):
            """Per-instruction closures projecting v token-chunk c."""
            ps = [None]

            def mk_mm(d):
                def go():
                    if d == 0:
                        ps[0] = psum.tile([128, GD], F32, tag="proj",
                                          name="ps_v")
                    nc.tensor.matmul(
                        ps[0], ht_bf[:, d, c * 128:(c + 1) * 128],
                        wv_bf[:, d, :], start=(d == 0), stop=False)
                return go

            def bias_mm():
                nc.tensor.matmul(ps[0], ones_sb, bv_sb,
                                 start=False, stop=True)

            def copy():
                nc.vector.tensor_copy(
                    out=v_sb[:, c, :, 0:HD],
                    in_=ps[0].rearrange("p (h j) -> p h j", h=GH))
            return [mk_mm(d) for d in range(DC)] + [bias_mm, copy]

        # ---- preamble: k_t0/q_t0 + first 6 v chunks, d-outer so the
        # matmuls pace with the arriving ht/w DMA chunks ----
        VPRE = 6
        kq_k = emit_kq_tile("k", 0, 0)
        kq_q = emit_kq_tile("q", 0, 0)
        with tc.tile_pool(name="vpre", bufs=VPRE, space="PSUM") as vpre:
            vps = [None] * VPRE
            for d in range(DC):
                kq_k[d]()
                kq_q[d]()
                for c in range(VPRE):
                    if d == 0:
                        vps[c] = vpre.tile([128, GD], F32, tag="vpre",
                                           name="vps")
                    nc.tensor.matmul(
                        vps[c], ht_bf[:, d, c * 128:(c + 1) * 128],
                        wv_bf[:, d, :], start=(d == 0), stop=False)
            kq_k[DC]()  # bias-add copies
            kq_q[DC]()
            for c in range(VPRE):
                nc.tensor.matmul(vps[c], ones_sb, bv_sb,
                                 start=False, stop=True)
                nc.vector.tensor_copy(
                    out=v_sb[:, c, :, 0:HD],
                    in_=vps[c].rearrange("p (h j) -> p h j", h=GH))

        # ---- (g0, qt0): scores + exp only; ctx deferred to group 3.
        # Background: remaining kT tiles (needed by qt0's own scores),
        # remaining v chunks, remaining qT tiles. ----
        with tc.tile_pool(name="sc0_psum", bufs=2, space="PSUM") as sc0_psum:
            bg0 = []
            for t in range(1, 4):
                bg0.extend(emit_kq_tile("k", 0, t))
            for c in range(VPRE, KC):
                bg0.extend(v_chunk_items(c))
            for t in range(1, 4):
                bg0.extend(emit_kq_tile("q", 0, t))
            bg0_i = [0]
            for c in range(KC):
                sc = sc0_psum.tile([128, 1024], F32, tag="sc", name="sc")
                for h01 in range(2):
                    nc.tensor.matmul(
                        sc[:, h01 * 512:(h01 + 1) * 512],
                        kT[h01 * 64:(h01 + 1) * 64, 0, c * 128:(c + 1) * 128],
                        qT[h01 * 64:(h01 + 1) * 64, 0, 0:512],
                        start=True, stop=True)
                nc.scalar.activation(
                    out=pr_hold[:, c, :], in_=sc,
                    func=mybir.ActivationFunctionType.Exp,
                    bias=mask_sb[:, c:c + 1], scale=SCALE)
                want = (len(bg0) - bg0_i[0] + (KC - 1 - c)) // max(KC - c, 1)
                for _ in range(want):
                    if bg0_i[0] < len(bg0):
                        bg0[bg0_i[0]]()
                        bg0_i[0] += 1
            while bg0_i[0] < len(bg0):
                bg0[bg0_i[0]]()
                bg0_i[0] += 1

        def deferred_qt0_items(out_t0, dctx):
            """ctx + epilogue for (g0, qt0), consuming pr_hold and the two
            proj-pool psum slots (free during group 3)."""
            items = []

            def mk_mm(c, h01, s_):
                def go():
                    if c == 0 and s_ == 0:
                        dctx[h01] = psum.tile([128, 512], F32, tag="proj",
                                              name="dctx")
                    nc.tensor.matmul(
                        dctx[h01][:, s_ * 65:s_ * 65 + 65],
                        pr_hold[:, c, h01 * 512 + s_ * 128:
                                h01 * 512 + (s_ + 1) * 128],
                        v_sb[:, c, h01, :],
                        start=(c == 0 and s_ == 0),
                        stop=(c == KC - 1 and s_ == 3))
                return go

            for c in range(KC):
                for h01 in range(2):
                    for s_ in range(4):
                        items.append(mk_mm(c, h01, s_))

            def mk_epi(h01):
                def go():
                    cps = dctx[h01][:, 0:4 * (HD + 1)]
                    rec = small.tile([128, 4], F32, tag="rec", name="rec")
                    nc.vector.reciprocal(
                        rec,
                        cps.rearrange("p (s x) -> p s x", x=HD + 1)[:, :, HD])
                    for s_ in range(4):
                        nc.vector.tensor_scalar_mul(
                            out=out_t0[:, s_, h01 * HD:(h01 + 1) * HD],
                            in0=cps[:, s_ * 65:s_ * 65 + HD],
                            scalar1=rec[:, s_:s_ + 1])
                return go
            items.extend([mk_epi(0), mk_epi(1)])

            def dma_out0():
                # only group 0's columns of qt0 remain; the rest streamed out
                # from the per-(qt, g) epilogues
                nc.sync.dma_start(
                    out=out_d[:].rearrange("(t s p) j -> t p s j",
                                           s=4, p=128)[0][:, :, 0:2 * HD],
                    in_=out_t0[:, :, 0:2 * HD])
            items.append(dma_out0)
            return items

        def bg_for_group(g, out_ts, dctx):
            items = []
            if g < 3:
                for which in ("k", "q"):
                    for t in range(4):
                        items.extend(emit_kq_tile(which, g + 1, t))
            else:
                items.extend(deferred_qt0_items(out_ts[0], dctx))
            return items

        # ---- attention ----
        with (
            tc.tile_pool(name="sc_psum", bufs=2, space="PSUM") as sc_psum,
            tc.tile_pool(name="ctx_psum", bufs=1, space="PSUM") as ctx_psum,
        ):
            out_view = out_d[:].rearrange("(t s p) j -> t p s j", s=4, p=128)
            out_ts = {}
            for qt in range(QT):
                out_ts[qt] = outp.tile([128, 4, GD], F32,
                                       tag=f"out{qt}", name="out_t")
            dctx = [None, None]
            for g in range(4):
                bg = bg_for_group(g, out_ts, dctx)
                bg_i = [0]
                qts = range(1, QT) if g == 0 else range(QT)
                n_steps = [len(qts) * KC]

                def pop_bg():
                    # aim to finish the queue ~8 steps before the group ends
                    n_steps[0] -= 1
                    eff = max(n_steps[0] - 8, 1) if n_steps[0] > 0 else 0
                    want = len(bg) - bg_i[0] if eff == 0 else (
                        (len(bg) - bg_i[0] + eff - 1) // eff)
                    for _ in range(want):
                        if bg_i[0] < len(bg):
                            bg[bg_i[0]]()
                            bg_i[0] += 1

                for qt in qts:
                    out_t = out_ts[qt]
                    ctx0 = ctx_psum.tile([128, 4 * (HD + 1)], F32, tag="ctx0",
                                         name="ctx0")
                    ctx1 = ctx_psum.tile([128, 4 * (HD + 1)], F32, tag="ctx1",
                                         name="ctx1")
                    ctxs = (ctx0, ctx1)
                    prs = [None] * KC

                    def emit_ctx(c):
                        for h01 in range(2):
                            cps = ctxs[h01]
                            for s_ in range(4):
                                nc.tensor.matmul(
                                    cps[:, s_ * 65:s_ * 65 + 65],
                                    prs[c][:, h01 * 512 + s_ * 128:
                                           h01 * 512 + (s_ + 1) * 128],
                                    v_sb[:, c, 2 * g + h01, :],
                                    start=(c == 0 and s_ == 0),
                                    stop=(c == KC - 1 and s_ == 3))

                    for c in range(KC):
                        sc = sc_psum.tile([128, 1024], F32, tag="sc", name="sc")
                        for h01 in range(2):
                            nc.tensor.matmul(
                                sc[:, h01 * 512:(h01 + 1) * 512],
                                kT[h01 * 64:(h01 + 1) * 64, g,
                                   c * 128:(c + 1) * 128],
                                qT[h01 * 64:(h01 + 1) * 64, g,
                                   qt * 512:(qt + 1) * 512],
                                start=True, stop=True)
                        pr = probs.tile([128, 1024], BF16, tag="pr", name="pr")
                        nc.scalar.activation(
                            out=pr, in_=sc,
                            func=mybir.ActivationFunctionType.Exp,
                            bias=mask_sb[:, c:c + 1], scale=SCALE)
                        prs[c] = pr
                        if c >= 1:
                            emit_ctx(c - 1)  # overlap ctx(c-1) with exp(c)
                        pop_bg()
                    emit_ctx(KC - 1)

                    for h01 in range(2):
                        cps = ctxs[h01]
                        rec = small.tile([128, 4], F32, tag="rec", name="rec")
                        nc.vector.reciprocal(
                            rec,
                            cps.rearrange("p (s x) -> p s x", x=HD + 1)[:, :, HD])
                        for s_ in range(4):
                            nc.vector.tensor_scalar_mul(
                                out=out_t[:, s_,
                                          (2 * g + h01) * HD:
                                          (2 * g + h01 + 1) * HD],
                                in0=cps[:, s_ * 65:s_ * 65 + HD],
                                scalar1=rec[:, s_:s_ + 1])
                    # stream this group's output columns out right away
                    gsl = slice(2 * g * HD, (2 * g + 2) * HD)
                    nc.sync.dma_start(out=out_view[qt][:, :, gsl],
                                      in_=out_t[:, :, gsl])
                while bg_i[0] < len(bg):  # flush
                    bg[bg_i[0]]()
                    bg_i[0] += 1


def _build():
    nc = bacc.Bacc()
    ht_d = nc.declare_dram_parameter("ht", [D, S], BF16, isOutput=False)
    wq_d = nc.declare_dram_parameter("wq", [D, GD], BF16, isOutput=False)
    wk_d = nc.declare_dram_parameter("wk", [D, GD], BF16, isOutput=False)
    wv_d = nc.declare_dram_parameter("wv", [D, GD], BF16, isOutput=False)
    bq_d = nc.declare_dram_parameter("bq", [128, 4], F32, isOutput=False)
    bk_d = nc.declare_dram_parameter("bk", [128, 4], F32, isOutput=False)
    bv_d = nc.declare_dram_parameter("bv", [1, GD], BF16, isOutput=False)
    mask_d = nc.declare_dram_parameter("mask", [128, KC], F32, isOutput=False)
    out_d = nc.declare_dram_parameter("out", [S, GD], F32, isOutput=True)
    with tile.TileContext(nc) as tc:
        _emit(tc, ht_d, wq_d, wk_d, wv_d, bq_d, bk_d, bv_d, mask_d, out_d)
    nc.compile()
    return nc


_NC = None


def _patch_ldw_opt():
    """Flip walrus's --enable-ldw-opt to true (experimental: overlaps
    LDWEIGHTS with matmuls).  Opt-in via BASS_LDW_OPT=1."""
    import os
    if os.environ.get("BASS_LDW_OPT") != "1":
        return
    import concourse.bass_utils as bu
    if getattr(bu, "_ldw_patched", False):
        return
    orig = bu.run_command

    def patched(argv, **kwargs):
        argv = ["--enable-ldw-opt=true" if a == "--enable-ldw-opt=false" else a
                for a in argv]
        return orig(argv, **kwargs)

    bu.run_command = patched
    bu._ldw_patched = True


def _get_nc():
    global _NC
    if _NC is None:
        _patch_ldw_opt()
        _NC = _build()
    return _NC


def _prep_in_maps(hidden_states, attention_mask, Wq, bq, Wk, bk, Wv, bv):
    import ml_dtypes
    bf16 = ml_dtypes.bfloat16

    hs = np.asarray(hidden_states, dtype=np.float32)
    am = np.asarray(attention_mask, dtype=np.float32)
    Wq = np.asarray(Wq, dtype=np.float32)
    Wk = np.asarray(Wk, dtype=np.float32)
    Wv = np.asarray(Wv, dtype=np.float32)
    bq = np.asarray(bq, dtype=np.float32)
    bk = np.asarray(bk, dtype=np.float32)
    bv = np.asarray(bv, dtype=np.float32)

    hts = [np.ascontiguousarray(hs[b].T).astype(bf16) for b in range(B)]
    masks = [np.ascontiguousarray(am[b, 0, 0].reshape(KC, 128).T)
             for b in range(B)]
    in_maps = []
    for c in range(NCORES):
        b, g = divmod(c, 2)
        sl = slice(g * GD, (g + 1) * GD)
        in_maps.append({
            "ht": hts[b],
            "wq": np.ascontiguousarray(Wq[:, sl]).astype(bf16),
            "wk": np.ascontiguousarray(Wk[:, sl]).astype(bf16),
            "wv": np.ascontiguousarray(Wv[:, sl]).astype(bf16),
            "bq": np.ascontiguousarray(bq[sl].reshape(4, 128).T),
            "bk": np.ascontiguousarray(bk[sl].reshape(4, 128).T),
            "bv": np.ascontiguousarray(bv[sl].reshape(1, GD)).astype(bf16),
            "mask": masks[b],
        })
    return in_maps


def _install_trace_hooks():
    """Make trace=True work in this container: register the NTFF profile
    hook under the name bass_utils imports, and keep artifacts local."""
    import sys
    import types

    if "antenv.axon_hooks" not in sys.modules:
        mod = types.ModuleType("antenv.axon_hooks")
        mod._hook = None

        def set_axon_ntff_profile_hook(h):
            mod._hook = h

        def get_axon_ntff_profile_hook():
            return mod._hook

        mod.set_axon_ntff_profile_hook = set_axon_ntff_profile_hook
        mod.get_axon_ntff_profile_hook = get_axon_ntff_profile_hook
        sys.modules["antenv.axon_hooks"] = mod
        try:
            from trn_agent_boot.trn_boot import _ntff_profile_via_ctypes
            mod._hook = _ntff_profile_via_ctypes("/opt/axon/libaxon_pjrt.so")
        except Exception as e:  # profiling degrades, run still works
            print(f"ntff hook install failed: {e}")
    import concourse.bass_utils as bu
    bu.upload_artifacts = lambda tmpdir: tmpdir


def run(inputs, trace=False, trace_cores=None):
    """Run the SPMD kernel.  Returns (full_output, exec_time_ns_or_None)."""
    if trace:
        _install_trace_hooks()
    nc = _get_nc()
    in_maps = _prep_in_maps(**inputs)
    res = run_bass_kernel_spmd(
        nc, in_maps, core_ids=list(range(NCORES)), trace=trace,
        **({"trace_cores": trace_cores} if trace_cores is not None else {}),
    )
    out = np.empty((B, S, D), np.float32)
    for c in range(NCORES):
        b, g = divmod(c, 2)
        out[b, :, g * GD:(g + 1) * GD] = res.results[c]["out"]
    return out, res.exec_time_ns


def kernel(hidden_states, attention_mask, Wq, bq, Wk, bk, Wv, bv):
    out, _ = run(dict(
        hidden_states=hidden_states, attention_mask=attention_mask,
        Wq=Wq, bq=bq, Wk=Wk, bk=bk, Wv=Wv, bv=bv,
    ))
    return out
